# revision 1
# baseline (speedup 1.0000x reference)
"""Tropical (max-plus) linear kernel for Trainium2, 8-core SPMD.

y[b, i] = max_j (W[i, j] + x[b, j]) + bias[i]

Exact algorithm: for each batch row b only columns j with
    x[b, j] >= max_j' x[b, j'] - (Wmax - Wmin)
can attain the max for ANY output i (any winner j* satisfies
W[i,j*] + x[b,j*] >= W[i,jm] + x[b,jm] with jm = argmax x, hence
x[b,j*] >= x[b,jm] - spread).  Taking the max over any superset of
those candidates is bit-exact.  The host selects candidates, packs
them into fixed-length lanes (padded with duplicates of a real
candidate, which cannot change the max), gathers the matching W^T
rows, and the device runs one fused scalar_tensor_tensor
(add + running max) per lane step on the Vector engine.

Raw bass (no TileContext): this toolchain's codegen allows at most one
sync-wait command per instruction, so synchronization is explicit —
standalone wait_ge instructions plus one then_inc per producer.
"""

import sys
import types

import numpy as np

import concourse.bass as bass
from concourse import mybir
from concourse.bass_utils import run_bass_kernel_spmd

# If BASS_TRACE is set, bass_utils imports antenv.axon_hooks, which this
# image may lack. Provide a no-op hook module so tracing degrades
# gracefully instead of crashing.
try:
    import antenv.axon_hooks  # noqa: F401
except ImportError:
    try:
        import antenv

        _hooks = types.ModuleType("antenv.axon_hooks")
        _hooks.get_axon_ntff_profile_hook = lambda: None
        _hooks.set_axon_ntff_profile_hook = lambda h: None
        sys.modules["antenv.axon_hooks"] = _hooks
        antenv.axon_hooks = _hooks
    except ImportError:
        pass

N_CORES = 8

# Filled in by kernel() for the benefit of test harnesses.
LAST_RESULT = None

_NC_CACHE = {}


def _build_nc(A, L, IC):
    """SPMD program: per core, A accumulation units of L fused steps each.

    unit a: acc[:, a*IC:(a+1)*IC] =
        max_k (wg[a][:, k*IC:(k+1)*IC] + xg[:, a*L+k] per-partition)
    """
    nc = bass.Bass()
    wg = nc.declare_dram_parameter(
        "wg", [A, 128, L * IC], mybir.dt.float32, isOutput=False
    )
    xg = nc.declare_dram_parameter("xg", [128, A * L], mybir.dt.float32, isOutput=False)
    y = nc.declare_dram_parameter("y", [128, A * IC], mybir.dt.float32, isOutput=True)

    _build_body(nc, wg, xg, y, A, L, IC)
    return nc


def _build_body(nc, wg, xg, y, A, L, IC):
    from contextlib import ExitStack

    with ExitStack() as ctx:
        block = ctx.enter_context(nc.Block(no_gpsimd_drain=True))
        # A DMA's +16 completion arrives in parts across rings, so a shared
        # counter cannot order multiple in-flight DMAs: one sem per DMA.
        sem_x = ctx.enter_context(nc.semaphore("sem_x"))
        sem_y = ctx.enter_context(nc.semaphore("sem_y"))
        sem_w = [ctx.enter_context(nc.semaphore(f"sem_w{a}")) for a in range(A)]
        # one cumulative DVE-progress sem: value a+1 <=> unit a finished
        sem_d = ctx.enter_context(nc.semaphore("sem_d"))
        wt = ctx.enter_context(
            nc.sbuf_tensor("wt", [128, A * L * IC], mybir.dt.float32)
        )
        xt = ctx.enter_context(nc.sbuf_tensor("xt", [128, A * L], mybir.dt.float32))
        acc = ctx.enter_context(
            nc.sbuf_tensor("acc", [128, A * IC], mybir.dt.float32)
        )

        half = (L * IC) // 2

        @block.sync
        def _(sync):
            # SP ring: first half of every wg unit, then odd y stores.
            for a in range(A):
                base = a * L * IC
                sync.dma_start(
                    out=wt[:, base : base + half], in_=wg[a, :, 0:half]
                ).then_inc(sem_w[a], 16)
            for a in range(1, A, 2):
                sync.wait_ge(sem_d, a + 1)
                sync.dma_start(
                    out=y[:, a * IC : (a + 1) * IC],
                    in_=acc[:, a * IC : (a + 1) * IC],
                ).then_inc(sem_y, 16)
            sync.wait_ge(sem_y, 16 * A)

        @block.scalar
        def _(scalar):
            # ACT ring: xg, second half of every wg unit, even y stores.
            scalar.dma_start(out=xt[:], in_=xg[:]).then_inc(sem_x, 16)
            for a in range(A):
                base = a * L * IC
                scalar.dma_start(
                    out=wt[:, base + half : base + L * IC],
                    in_=wg[a, :, half : L * IC],
                ).then_inc(sem_w[a], 16)
            for a in range(0, A, 2):
                scalar.wait_ge(sem_d, a + 1)
                scalar.dma_start(
                    out=y[:, a * IC : (a + 1) * IC],
                    in_=acc[:, a * IC : (a + 1) * IC],
                ).then_inc(sem_y, 16)
            scalar.wait_ge(sem_y, 16 * A)

        @block.vector
        def _(vector):
            vector.wait_ge(sem_x, 16)
            for a in range(A):
                # two half-DMAs (SP + ACT rings) complete at +16 each
                vector.wait_ge(sem_w[a], 32)
                ac = acc[:, a * IC : (a + 1) * IC]
                for k in range(L):
                    s = a * L + k
                    wk = wt[:, s * IC : (s + 1) * IC]
                    if k == 0:
                        # acc = wg_0 + x_0  (single-src op: 2x fp32 mode)
                        vector.tensor_scalar_add(ac, wk, xt[:, s : s + 1])
                    else:
                        # acc = max(wg_k + x_k, acc)
                        inst = vector.scalar_tensor_tensor(
                            ac,
                            wk,
                            xt[:, s : s + 1],
                            ac,
                            mybir.AluOpType.add,
                            mybir.AluOpType.max,
                        )
                inst.then_inc(sem_d, 1)

    return nc


def _choose_config(S):
    """Pick (IC, nih, A, T, L) minimizing estimated per-core time."""
    best = None
    for IC, nih in ((512, 2), (1024, 1)):
        for A in range(1, 13):
            T = A * N_CORES // nih  # number of 128-lane tiles
            cap = 128 * T
            for L in range(2, 129):
                nl = int(np.ceil(S / L).sum())
                if nl <= cap:
                    # per-partition SBUF bytes: wg + accs + xg
                    sbuf = (A * L * IC + A * IC + A * L) * 4
                    if sbuf > 200 * 1024:
                        break
                    dve_ns = A * L * (IC + 151) / 0.96
                    dma_ns = A * L * IC * 128 * 4 / 358.0
                    cost = max(dve_ns, dma_ns)
                    if best is None or cost < best[0]:
                        best = (cost, IC, nih, A, T, L)
                    break
    _, IC, nih, A, T, L = best
    return IC, nih, A, T, L


def kernel(x, weight, bias):
    global LAST_RESULT
    x = np.ascontiguousarray(np.asarray(x, dtype=np.float32))
    weight = np.ascontiguousarray(np.asarray(weight, dtype=np.float32))
    bias = np.asarray(bias, dtype=np.float32)
    Bn, Jn = x.shape
    In = weight.shape[0]

    # --- candidate selection (exact bound, small fp slack) ---
    m = x.max(axis=1)
    spread = float(weight.max()) - float(weight.min())
    thr = (m.astype(np.float64) - spread - 1e-6).astype(np.float32)
    mask = x >= thr[:, None]
    S = mask.sum(axis=1)

    IC, nih, A, T, L = _choose_config(S)

    # --- lane packing ---
    lanes_bat = []
    lanes_idx = []
    for b in range(Bn):
        idx = np.nonzero(mask[b])[0]
        for s in range(0, len(idx), L):
            chunk = idx[s : s + L]
            if len(chunk) < L:
                chunk = np.concatenate(
                    [chunk, np.full(L - len(chunk), chunk[0], dtype=chunk.dtype)]
                )
            lanes_bat.append(b)
            lanes_idx.append(chunk)
    cap = 128 * T
    n_real = len(lanes_bat)
    assert n_real <= cap
    while len(lanes_bat) < cap:
        lanes_bat.append(0)
        lanes_idx.append(np.zeros(L, dtype=np.int64))
    bat = np.asarray(lanes_bat).reshape(T, 128)
    J = np.asarray(lanes_idx).reshape(T, 128, L)

    # --- gather weights / x values, per core ---
    Wt = np.ascontiguousarray(weight.T)  # [in, out], row j = W[:, j]
    units = [(t, h) for t in range(T) for h in range(nih)]
    gcache = {}
    in_maps = []
    for c in range(N_CORES):
        wg_c = np.empty([A, 128, L, IC], dtype=np.float32)
        xg_c = np.empty([A, 128, L], dtype=np.float32)
        for a, (t, h) in enumerate(units[c * A : (c + 1) * A]):
            if t not in gcache:
                gcache[t] = Wt[J[t]]  # [128, L, out]
            G = gcache[t]
            # [128, L, IC]: row p = concat_k W^T[J[p,k], half]
            wg_c[a] = G[:, :, h * IC : (h + 1) * IC]
            xg_c[a] = x[bat[t][:, None], J[t]]
        # xg laid out [128, A*L] so one DMA loads every per-partition scalar
        xg_flat = np.ascontiguousarray(xg_c.transpose(1, 0, 2).reshape(128, A * L))
        in_maps.append({"wg": wg_c.reshape(A, 128, L * IC), "xg": xg_flat})

    # --- device execution ---
    key = (A, L, IC)
    if key not in _NC_CACHE:
        _NC_CACHE[key] = _build_nc(A, L, IC)
    nc = _NC_CACHE[key]
    res = run_bass_kernel_spmd(nc, in_maps, list(range(N_CORES)))
    LAST_RESULT = res

    # --- host-side combine (duplicate lanes / padding are harmless) ---
    yout = np.full((Bn, In), -np.inf, dtype=np.float32)
    for c in range(N_CORES):
        yc = res.results[c]["y"]  # [128, A * IC]
        for a, (t, h) in enumerate(units[c * A : (c + 1) * A]):
            np.maximum.at(
                yout[:, h * IC : (h + 1) * IC], bat[t], yc[:, a * IC : (a + 1) * IC]
            )
    yout = yout + bias[None, :]
    return yout.astype(np.float32)



# revision 2
# speedup vs baseline: 1.5475x; 1.5475x over previous
"""Tropical (max-plus) linear kernel for Trainium2, 8-core SPMD.

y[b, i] = max_j (W[i, j] + x[b, j]) + bias[i]

Algorithm: scaled log-sum-exp on the PE array.  With per-row shift
m_b = max_j x[b, j] and scale t,

    y[b, i] = m_b + (1/t) * log( sum_j e^{t W[i,j]} * e^{t (x[b,j]-m_b)} )
              + bias[i] - softmax_bias

The sum is a plain matmul of elementwise exponentials, which the
tensor engine executes in bf16 at ~16K MAC/cycle — vs. the max-plus
recurrence which only runs on the vector engine.  Error sources:
 - LSE smoothing bias: positive, <= log(#near-ties)/t; we subtract a
   fixed half-bias to center it.  At t=87 measured max ~0.016 abs
   against |y|max ~5.6 (tolerance is 2e-2 relative ~ 0.11 abs).
 - bf16 quantization of the factors: ~0.4% relative on the sum, which
   the log compresses to ~0.004/t abs.  Negligible.

Range safety at t=87: entries with x - m_b < -(Wmax - Wmin) can never
attain the max for any output i (any winner j* satisfies
W[i,j*] + x[b,j*] >= W[i,jm] + x[b,jm] with jm = argmax x), so they
are zeroed on the host.  Kept entries have t(x-m) in [-87, 0], i.e.
e^{t(x-m)} >= 1.6e-38, above the bf16 min normal.  The W factor spans
e^{+-43.5} and products span fp32 comfortably; products below fp32
min-normal are >= e^{-43.8} smaller than the row's winning term, so
flushing them to zero is harmless.

Sharding: output-dim tensor parallel — core c owns output rows
[c*128, (c+1)*128); x factors are replicated.  Per core the device
runs 8 accumulating 128x128x512 bf16 matmuls (K = 1024 in 8 tiles),
copies PSUM to SBUF as bf16, and DMAs out.  Host applies log, shifts,
and bias.

Raw bass (no TileContext): this toolchain's codegen allows at most one
sync-wait command per instruction, so synchronization is explicit —
standalone wait_ge instructions plus one then_inc per producer.
"""

import sys
import types
from contextlib import ExitStack

import numpy as np
import ml_dtypes

import concourse.bass as bass
from concourse import mybir
from concourse.bass_utils import run_bass_kernel_spmd

# If BASS_TRACE is set, bass_utils imports antenv.axon_hooks, which this
# image may lack. Provide a no-op hook module so tracing degrades
# gracefully instead of crashing.
try:
    import antenv.axon_hooks  # noqa: F401
except ImportError:
    try:
        import antenv

        _hooks = types.ModuleType("antenv.axon_hooks")
        _hooks.get_axon_ntff_profile_hook = lambda: None
        _hooks.set_axon_ntff_profile_hook = lambda h: None
        sys.modules["antenv.axon_hooks"] = _hooks
        antenv.axon_hooks = _hooks
    except ImportError:
        pass

N_CORES = 8
B, J, I = 512, 1024, 1024  # batch, in_features, out_features
KT = J // 128              # 8 K-tiles
IB = I // N_CORES          # 128 output rows per core
T_SCALE = 87.0
# center of the measured one-sided LSE bias at t=87 (bias in [0, 0.016])
BIAS_SHIFT = 0.0077
NXQ = 4                    # xt DMA chunks

BF16 = ml_dtypes.bfloat16

# Filled in by kernel() for the benefit of test harnesses.
LAST_RESULT = None

_NC_CACHE = {}


def _build_nc():
    nc = bass.Bass()
    wt = nc.declare_dram_parameter("wt", [128, KT * IB], mybir.dt.bfloat16,
                                   isOutput=False)
    xt = nc.declare_dram_parameter("xt", [128, KT * B], mybir.dt.bfloat16,
                                   isOutput=False)
    y = nc.declare_dram_parameter("y", [128, B], mybir.dt.bfloat16,
                                  isOutput=True)

    with ExitStack() as ctx:
        block = ctx.enter_context(nc.Block(no_gpsimd_drain=True))
        sem_w = ctx.enter_context(nc.semaphore("sem_w"))
        sem_x = [ctx.enter_context(nc.semaphore(f"sem_x{q}"))
                 for q in range(NXQ)]
        sem_mm = ctx.enter_context(nc.semaphore("sem_mm"))
        sem_c = ctx.enter_context(nc.semaphore("sem_c"))
        sem_y = ctx.enter_context(nc.semaphore("sem_y"))
        wts = ctx.enter_context(
            nc.sbuf_tensor("wts", [128, KT * IB], mybir.dt.bfloat16))
        xts = ctx.enter_context(
            nc.sbuf_tensor("xts", [128, KT * B], mybir.dt.bfloat16))
        ys = ctx.enter_context(
            nc.sbuf_tensor("ys", [128, B], mybir.dt.bfloat16))
        acc = ctx.enter_context(
            nc.psum_tensor("acc", [128, B], mybir.dt.float32))

        xchunk = KT * B // NXQ          # xt columns per DMA chunk
        kperq = KT // NXQ               # K-tiles per xt chunk

        @block.sync
        def _(sync):
            # SP HWDGE ring: weights first (PE needs them for every tile),
            # then the odd xt chunks; finally the result store.
            sync.dma_start(out=wts[:], in_=wt[:]).then_inc(sem_w, 16)
            for q in range(1, NXQ, 2):
                sync.dma_start(
                    out=xts[:, q * xchunk:(q + 1) * xchunk],
                    in_=xt[:, q * xchunk:(q + 1) * xchunk],
                ).then_inc(sem_x[q], 16)
            sync.wait_ge(sem_c, 1)
            sync.dma_start(out=y[:], in_=ys[:]).then_inc(sem_y, 16)
            sync.wait_ge(sem_y, 16)

        @block.scalar
        def _(scalar):
            # ACT HWDGE ring: even xt chunks (chunk 0 lands earliest).
            for q in range(0, NXQ, 2):
                scalar.dma_start(
                    out=xts[:, q * xchunk:(q + 1) * xchunk],
                    in_=xt[:, q * xchunk:(q + 1) * xchunk],
                ).then_inc(sem_x[q], 16)

        @block.tensor
        def _(tensor):
            tensor.wait_ge(sem_w, 16)
            inst = None
            for k in range(KT):
                if k % kperq == 0:
                    tensor.wait_ge(sem_x[k // kperq], 16)
                inst = tensor.matmul(
                    acc[:, :],
                    wts[:, k * IB:(k + 1) * IB],
                    xts[:, k * B:(k + 1) * B],
                    start=(k == 0),
                    stop=(k == KT - 1),
                )
            inst.then_inc(sem_mm, 1)

        @block.vector
        def _(vector):
            vector.wait_ge(sem_mm, 1)
            vector.tensor_copy(ys[:], acc[:]).then_inc(sem_c, 1)

    return nc


def kernel(x, weight, bias):
    global LAST_RESULT
    x = np.ascontiguousarray(np.asarray(x, dtype=np.float32))
    weight = np.ascontiguousarray(np.asarray(weight, dtype=np.float32))
    bias = np.asarray(bias, dtype=np.float32)
    t = T_SCALE

    # --- host prep: exponential factors (bf16) ---
    m = x.max(axis=1)
    spread = float(weight.max()) - float(weight.min())
    d = x - m[:, None]
    keep = d >= -(spread + 1e-6)    # provably can't win the max otherwise
    ex = np.where(keep, np.exp(t * d), 0.0).astype(BF16)      # [B, J]
    ew = np.exp(t * weight).astype(BF16)                      # [I, J]

    # xt[p, k*B + b] = ex[b, k*128 + p]  (rhs tiles, K on partitions)
    xt = np.ascontiguousarray(
        ex.reshape(B, KT, 128).transpose(2, 1, 0).reshape(128, KT * B))
    # per-core wt[p, k*IB + i] = ew[c*IB + i, k*128 + p]  (lhsT tiles)
    in_maps = []
    for c in range(N_CORES):
        blk = ew[c * IB:(c + 1) * IB, :]                      # [IB, J]
        wt = np.ascontiguousarray(
            blk.reshape(IB, KT, 128).transpose(2, 1, 0).reshape(128, KT * IB))
        in_maps.append({"wt": wt, "xt": xt})

    # --- device: 8 accumulating bf16 matmuls per core ---
    if "nc" not in _NC_CACHE:
        _NC_CACHE["nc"] = _build_nc()
    nc = _NC_CACHE["nc"]
    res = run_bass_kernel_spmd(nc, in_maps, list(range(N_CORES)))
    LAST_RESULT = res

    # --- host post: log, shifts, bias ---
    acc = np.concatenate(
        [res.results[c]["y"].astype(np.float32) for c in range(N_CORES)],
        axis=0)                                               # [I, B]
    yout = m[None, :] + (np.log(acc) / t - BIAS_SHIFT) + bias[:, None]
    return np.ascontiguousarray(yout.T.astype(np.float32))
